# revision 1
# baseline (speedup 1.0000x reference)
"""Distributed exact inner-product top-k (brute-force kNN) on 8 TRN2 NeuronCores.

Sharding: codebook W is split row-wise into 8 shards of 25000 (one per core);
x is replicated.  Host pre-transposes both so the contraction dim (128) lands
on SBUF partitions.

Device kernel (SPMD, identical graph per core, no collectives needed):
  - per 512-wide vocab chunk: scores tile [128 rows, 512] = bf16 matmul into
    PSUM (f32 accumulation)
  - DVE max / max_index extract the chunk's top-8 values + positions
  - candidates (49 chunks x 8 = 392 per row per core) DMA'd out

Host merge (the all-gather + final top-k of the distributed ANN pattern):
  - exact f64 re-rank of the 8*392 = 3136 device-selected candidates per row
    (0.8 GFLOP on host vs 52 GFLOP of scoring on device) removes the bf16/
    fast-matmul selection noise entirely
  - final top-128 ordered like jax.lax.top_k (value desc, index asc)
  - exactness guard: a 512-chunk can hide a true top-128 element only if its
    8th-best device score clears the row's exact 128th value minus the score
    noise bound; such rows (expected ~0 per run for this data distribution)
    are recomputed exactly on host, as are rows with duplicated winners.
"""

import numpy as np

B = 1024
D = 128
VOCAB = 200000
NCORES = 8
VSHARD = VOCAB // NCORES  # 25000
CHUNK = 512
NCHUNK = (VSHARD + CHUNK - 1) // CHUNK  # 49 (last chunk is 424 wide)
NCAND = NCHUNK * 8  # 392
TOPK = 128

# Device scores use bf16 inputs (f32 accumulation): |device - exact| on scores
# of scale ~72 measured < 0.2; guard margin is ~2.5x that worst case.
SCORE_NOISE_BOUND = 0.5

LAST_RESULTS = None  # BassKernelResults of the most recent run (for profiling)
_CACHED_NC = None


def build_kernel():
    import concourse.bass as bass  # noqa: F401
    import concourse.tile as tile
    from concourse import bacc, mybir

    F32 = mybir.dt.float32
    BF16 = mybir.dt.bfloat16
    U32 = mybir.dt.uint32

    nc = bacc.Bacc("TRN2", target_bir_lowering=False, debug=False)
    wt_d = nc.dram_tensor("wt", [D, VSHARD], BF16, kind="ExternalInput")
    xt_d = nc.dram_tensor("xt", [D, B], BF16, kind="ExternalInput")
    vals_d = nc.dram_tensor("out_vals", [B, NCAND], F32, kind="ExternalOutput")
    idx_d = nc.dram_tensor("out_idx", [B, NCAND], U32, kind="ExternalOutput")

    with tile.TileContext(nc) as tc:
        with (
            tc.tile_pool(name="wt", bufs=1) as wt_pool,
            tc.tile_pool(name="xt", bufs=1) as xt_pool,
            tc.tile_pool(name="psum", bufs=8, space="PSUM") as psum_pool,
            tc.tile_pool(name="cand", bufs=2) as cand_pool,
        ):
            wt_sb = wt_pool.tile([D, VSHARD], BF16)
            xt_sb = xt_pool.tile([D, B], BF16)
            # xt first: the first matmul's stationary operand should not wait
            # behind the whole 6.4MB W load; 16 splits spread W across queues.
            nc.sync.dma_start(xt_sb[:], xt_d[:])
            nsplit = 16
            step = VSHARD // nsplit
            for s in range(nsplit):
                hi = VSHARD if s == nsplit - 1 else (s + 1) * step
                nc.sync.dma_start(wt_sb[:, s * step:hi], wt_d[:, s * step:hi])

            for g in range(B // 128):
                vals_sb = cand_pool.tile([128, NCAND], F32, tag="vals")
                idx_sb = cand_pool.tile([128, NCAND], U32, tag="idx")
                for c in range(NCHUNK):
                    w = min(CHUNK, VSHARD - c * CHUNK)
                    ps = psum_pool.tile([128, CHUNK], F32)
                    nc.tensor.matmul(
                        ps[:, :w],
                        xt_sb[:, g * 128:(g + 1) * 128],
                        wt_sb[:, c * CHUNK:c * CHUNK + w],
                        start=True, stop=True,
                    )
                    nc.vector.max(vals_sb[:, 8 * c:8 * c + 8], ps[:, :w])
                    nc.vector.max_index(
                        idx_sb[:, 8 * c:8 * c + 8],
                        vals_sb[:, 8 * c:8 * c + 8],
                        ps[:, :w],
                    )
                nc.sync.dma_start(vals_d[g * 128:(g + 1) * 128, :], vals_sb[:])
                nc.sync.dma_start(idx_d[g * 128:(g + 1) * 128, :], idx_sb[:])
    nc.compile()
    return nc


def _topk_rows(vals, gidx, k):
    """Per-row top-k ordered like jax.lax.top_k: value desc, index asc."""
    order = np.lexsort((gidx, -vals), axis=-1)[:, :k]
    return (
        np.take_along_axis(gidx, order, axis=1),
        np.take_along_axis(vals, order, axis=1),
    )


def kernel(x: np.ndarray, W: np.ndarray, topk) -> np.ndarray:
    global LAST_RESULTS, _CACHED_NC
    import os

    import ml_dtypes

    from concourse.bass_utils import run_bass_kernel_spmd

    assert x.shape == (B, D) and W.shape == (VOCAB, D)
    assert int(topk) == TOPK
    x = np.ascontiguousarray(np.asarray(x, dtype=np.float32))
    W = np.ascontiguousarray(np.asarray(W, dtype=np.float32))

    if _CACHED_NC is None:
        _CACHED_NC = build_kernel()
    nc = _CACHED_NC

    xt = np.ascontiguousarray(x.T).astype(ml_dtypes.bfloat16)
    in_maps = []
    for i in range(NCORES):
        wt_i = np.ascontiguousarray(
            W[i * VSHARD:(i + 1) * VSHARD].T
        ).astype(ml_dtypes.bfloat16)
        in_maps.append({"wt": wt_i, "xt": xt})

    LAST_RESULTS = run_bass_kernel_spmd(
        nc,
        in_maps,
        core_ids=list(range(NCORES)),
        trace=bool(int(os.environ.get("KERNEL_TRACE", "0"))),
    )
    results = LAST_RESULTS.results

    vals_all = np.concatenate(
        [results[i]["out_vals"] for i in range(NCORES)], axis=1
    ).astype(np.float64)  # [B, 8*392]
    idx_local = np.concatenate(
        [results[i]["out_idx"].astype(np.int64) for i in range(NCORES)], axis=1
    )
    # per-chunk local index -> global vocab index
    chunk_base = np.concatenate(
        [i * VSHARD + CHUNK * (np.arange(NCAND) // 8) for i in range(NCORES)]
    ).astype(np.int64)  # [8*392]
    gidx_all = np.clip(idx_local, 0, CHUNK - 1) + chunk_base[None, :]
    bad_idx_rows = (idx_local >= CHUNK).any(axis=1)

    # Exact re-rank of device-selected candidates: f64 inner products.
    x64 = x.astype(np.float64)
    W64 = W.astype(np.float64)
    exact = np.empty_like(vals_all)
    STEP = 128
    for r0 in range(0, B, STEP):
        r1 = r0 + STEP
        gW = W64[gidx_all[r0:r1]]  # [STEP, ncand, D]
        exact[r0:r1] = np.einsum("bjd,bd->bj", gW, x64[r0:r1])

    gidx_top, vals_top = _topk_rows(exact, gidx_all, TOPK)

    # Exactness guard + fallback.
    t_row = vals_top[:, -1]  # [B] exact 128th value
    chunk_min = vals_all.reshape(B, -1, 8)[:, :, 7]
    risky = (chunk_min >= (t_row[:, None] - SCORE_NOISE_BOUND)).any(axis=1)
    idx_chunks = np.sort(gidx_all.reshape(B, -1, 8), axis=2)
    dup = (np.diff(idx_chunks, axis=2) == 0).any(axis=(1, 2))
    for r in np.flatnonzero(risky | dup | bad_idx_rows):
        s = x64[r] @ W64.T
        gidx_top[r] = np.lexsort((np.arange(VOCAB), -s))[:TOPK]

    return gidx_top.astype(np.int32)



# revision 2
# speedup vs baseline: 2.0427x; 2.0427x over previous
"""Distributed exact inner-product top-k (brute-force kNN) on 8 TRN2 NeuronCores.

Sharding: codebook W is split row-wise into 8 shards of 25000 (one per core);
x is replicated.  Host pre-transposes both so the contraction dim (128) lands
on SBUF partitions.

Device kernel (SPMD, identical graph per core, no collectives):
  - per 2048-wide vocab region (4 PSUM banks): 4x bf16 matmuls [128 rows, 512]
    into PSUM (f32 accumulation)
  - the region is then reduced to 512 window-maxima (window = 4 vocab columns)
    by one of three engine routes, so the reduction work is spread across the
    DVE, Activation and Pool engines which all run concurrently:
      A: DVE windowed tensor_reduce(max)      [128,512,4] -> [128,512]
      B: Act copy PSUM->SBUF bf16, then two DVE bf16 folds (2x mode)
      C: Pool pairwise max PSUM halves -> bf16, then one DVE bf16 fold
  - window maxima (bf16) DMA'd out: [1024, 6250] per core

Host merge (the all-gather + final top-k of the distributed ANN pattern):
  - per row, select every window whose max clears (128th-largest window max
    - MARGIN); gather those windows' 4 member columns as candidates
  - exact f64 re-rank of the candidates; final top-128 ordered like
    jax.lax.top_k (value desc, index asc)
  - exactness guard: MARGIN >= 2*EPS guarantees containment of the true
    top-128 given |device window max - exact window max| <= EPS; EPS is
    validated per-run on every selected window (device value vs exact f64
    value), and violating rows (expected none) are recomputed exactly.
"""

import numpy as np

B = 1024
D = 128
VOCAB = 200000
NCORES = 8
VSHARD = VOCAB // NCORES  # 25000
REGION = 2048  # 4 PSUM banks of f32
NREG = 12  # full regions per shard
TAIL = VSHARD - NREG * REGION  # 424
WINDOW = 4
NWIN = VSHARD // WINDOW  # 6250 window maxima per core per row
TOPK = 128

# Engine route per full region (tail is always 'A'):
#   A = DVE windowed reduce, B = Act copy + DVE folds, C = Pool pair + DVE fold
ROUTES = "AAAAAAAAAAAA"

# |device window max - exact window max| bound: bf16 matmul noise (~0.1) +
# bf16 output quantization (~0.2 at score ~45).  Validated at runtime.
EPS_BOUND = 0.5
MARGIN = 1.2  # >= 2*EPS_BOUND + slack

LAST_RESULTS = None  # BassKernelResults of the most recent run (for profiling)
_CACHED_NC = None


def build_kernel():
    import concourse.bass as bass  # noqa: F401
    import concourse.tile as tile
    from concourse import bacc, mybir

    F32 = mybir.dt.float32
    BF16 = mybir.dt.bfloat16
    AX = mybir.AxisListType.X
    MAX = mybir.AluOpType.max
    COPY = mybir.ActivationFunctionType.Copy

    nc = bacc.Bacc("TRN2", target_bir_lowering=False, debug=False)
    wt_d = nc.dram_tensor("wt", [D, VSHARD], BF16, kind="ExternalInput")
    xt_d = nc.dram_tensor("xt", [D, B], BF16, kind="ExternalInput")
    out_d = nc.dram_tensor("out_win", [B, NWIN], BF16, kind="ExternalOutput")

    with tile.TileContext(nc) as tc:
        with (
            tc.tile_pool(name="wt", bufs=1) as wt_pool,
            tc.tile_pool(name="xt", bufs=1) as xt_pool,
            tc.tile_pool(name="psum", bufs=2, space="PSUM") as psum_pool,
            tc.tile_pool(name="outw", bufs=2) as out_pool,
            tc.tile_pool(name="ac", bufs=2) as ac_pool,
            tc.tile_pool(name="fold", bufs=2) as fold_pool,
            tc.tile_pool(name="po", bufs=2) as po_pool,
        ):
            wt_sb = wt_pool.tile([D, VSHARD], BF16)
            xt_sb = xt_pool.tile([D, B], BF16)
            # xt first: the first matmul's stationary operand should not wait
            # behind the whole 6.4MB W load; 16 splits spread W across queues.
            nc.sync.dma_start(xt_sb[:], xt_d[:])
            nsplit = 16
            step = VSHARD // nsplit
            for s in range(nsplit):
                hi = VSHARD if s == nsplit - 1 else (s + 1) * step
                nc.sync.dma_start(wt_sb[:, s * step:hi], wt_d[:, s * step:hi])

            for g in range(B // 128):
                out_sb = out_pool.tile([128, NWIN], BF16, tag="outw")
                xg = xt_sb[:, g * 128:(g + 1) * 128]
                for r in range(NREG + 1):
                    base = r * REGION
                    w_cols = REGION if r < NREG else TAIL
                    route = ROUTES[r] if r < NREG else "A"
                    ps = psum_pool.tile([128, REGION], F32)
                    for k in range(0, w_cols, 512):
                        kw = min(512, w_cols - k)
                        nc.tensor.matmul(
                            ps[:, k:k + kw],
                            xg,
                            wt_sb[:, base + k:base + k + kw],
                            start=True, stop=True,
                        )
                    owin = out_sb[:, base // 4:base // 4 + w_cols // 4]
                    if route == "A":
                        nc.vector.tensor_reduce(
                            owin,
                            ps[:, :w_cols].rearrange("p (n w) -> p n w", w=4),
                            axis=AX, op=MAX,
                        )
                    elif route == "B":
                        ac = ac_pool.tile([128, REGION], BF16, tag="ac")
                        nc.scalar.activation(ac[:], ps[:], COPY)
                        fd = fold_pool.tile([128, REGION // 2], BF16, tag="fold")
                        nc.vector.tensor_max(fd[:], ac[:, :1024], ac[:, 1024:])
                        nc.vector.tensor_max(owin, fd[:, :512], fd[:, 512:])
                    else:  # "C"
                        po = po_pool.tile([128, REGION // 2], BF16, tag="po")
                        nc.gpsimd.tensor_max(po[:], ps[:, :1024], ps[:, 1024:])
                        nc.vector.tensor_max(owin, po[:, :512], po[:, 512:])
                nc.sync.dma_start(out_d[g * 128:(g + 1) * 128, :], out_sb[:])
    nc.compile()
    return nc


def _build_colmap() -> np.ndarray:
    """[NWIN, 4] int64: window id -> the 4 shard-local vocab columns it covers."""
    cm = np.empty((NWIN, 4), np.int64)
    wbase = 0
    for r in range(NREG + 1):
        base = r * REGION
        n = (REGION if r < NREG else TAIL) // 4
        route = ROUTES[r] if r < NREG else "A"
        j = np.arange(n)[:, None]
        if route == "A":
            cols = base + 4 * j + np.arange(4)[None, :]
        else:  # B and C share the fold mapping {j, j+512, j+1024, j+1536}
            cols = base + j + np.array([0, 512, 1024, 1536])[None, :]
        cm[wbase:wbase + n] = cols
        wbase += n
    assert wbase == NWIN
    return cm


_COLMAP = _build_colmap()


def _topk_rows(vals, gidx, k):
    """Per-row top-k ordered like jax.lax.top_k: value desc, index asc."""
    order = np.lexsort((gidx, -vals), axis=-1)[:, :k]
    return (
        np.take_along_axis(gidx, order, axis=1),
        np.take_along_axis(vals, order, axis=1),
    )


def kernel(x: np.ndarray, W: np.ndarray, topk) -> np.ndarray:
    global LAST_RESULTS, _CACHED_NC
    import os

    import ml_dtypes

    from concourse.bass_utils import run_bass_kernel_spmd

    assert x.shape == (B, D) and W.shape == (VOCAB, D)
    assert int(topk) == TOPK
    x = np.ascontiguousarray(np.asarray(x, dtype=np.float32))
    W = np.ascontiguousarray(np.asarray(W, dtype=np.float32))

    if _CACHED_NC is None:
        _CACHED_NC = build_kernel()
    nc = _CACHED_NC

    xt = np.ascontiguousarray(x.T).astype(ml_dtypes.bfloat16)
    in_maps = []
    for i in range(NCORES):
        wt_i = np.ascontiguousarray(
            W[i * VSHARD:(i + 1) * VSHARD].T
        ).astype(ml_dtypes.bfloat16)
        in_maps.append({"wt": wt_i, "xt": xt})

    LAST_RESULTS = run_bass_kernel_spmd(
        nc,
        in_maps,
        core_ids=list(range(NCORES)),
        trace=bool(int(os.environ.get("KERNEL_TRACE", "0"))),
    )
    results = LAST_RESULTS.results

    # [B, 8*NWIN] device window maxima, f32
    wm = np.concatenate(
        [np.asarray(results[i]["out_win"]).astype(np.float32)
         for i in range(NCORES)], axis=1,
    )
    nwin_all = NCORES * NWIN

    # Per-row window selection: everything >= (128th-largest window max - MARGIN)
    kth = np.partition(wm, nwin_all - TOPK, axis=1)[:, nwin_all - TOPK]
    tau = kth - MARGIN
    counts = (wm >= tau[:, None]).sum(axis=1)
    K = int(min(max(int(counts.max()), TOPK + 64), 4096))
    topw = np.argpartition(-wm, K - 1, axis=1)[:, :K]  # [B, K] window ids

    core_id = topw // NWIN
    wi = topw % NWIN
    cand = (_COLMAP[wi] + core_id[..., None] * VSHARD).reshape(B, K * 4)

    # Exact f64 re-rank of the candidate columns.
    x64 = x.astype(np.float64)
    W64 = W.astype(np.float64)
    exact = np.empty((B, K * 4), np.float64)
    STEP = 128
    for r0 in range(0, B, STEP):
        r1 = r0 + STEP
        gW = W64[cand[r0:r1]]  # [STEP, K*4, D]
        exact[r0:r1] = np.einsum("bjd,bd->bj", gW, x64[r0:r1])

    gidx_top, vals_top = _topk_rows(exact, cand, TOPK)

    # Exactness guards.
    t128 = vals_top[:, -1]
    dev_wmax = np.take_along_axis(wm, topw, axis=1)
    true_wmax = exact.reshape(B, K, 4).max(axis=2)
    err = np.abs(dev_wmax - true_wmax).max(axis=1)
    bad = (
        (err > EPS_BOUND)
        | (tau + EPS_BOUND > t128)
        | (counts > K)
    )
    if os.environ.get("KERNEL_DEBUG"):
        print(f"[kernel] K={K} counts max={counts.max()} "
              f"err max={err.max():.4f} bad rows={int(bad.sum())}")
    for r in np.flatnonzero(bad):
        s = x64[r] @ W64.T
        gidx_top[r] = np.lexsort((np.arange(VOCAB), -s))[:TOPK]

    return gidx_top.astype(np.int32)


# revision 7
# speedup vs baseline: 2.6043x; 1.2749x over previous
"""Distributed exact inner-product top-k (brute-force kNN) on 8 TRN2 NeuronCores.

Sharding: codebook W is split row-wise into 8 shards of 25000 (one per core);
x is replicated.  Host pre-transposes both so the contraction dim (128) lands
on SBUF partitions.

Device kernel (SPMD, identical graph per core, no collectives):
  - per 2048-wide vocab region (4 PSUM banks): 4x bf16 matmuls [128 rows, 512]
    into PSUM (f32 accumulation)
  - the region is then reduced to 512 window-maxima (window = 4 vocab columns)
    by one of three engine routes, so the reduction work is spread across the
    DVE, Activation and Pool engines which all run concurrently:
      A: DVE windowed tensor_reduce(max)      [128,512,4] -> [128,512]
      B: Act copy PSUM->SBUF bf16, then two DVE bf16 folds (2x mode)
      C: Act copy PSUM->SBUF bf16, Pool bf16 fold, then one DVE bf16 fold
    (GPSIMD/Pool cannot read PSUM on TRN2, so Act does every PSUM->SBUF copy)
  - window maxima (bf16) DMA'd out: [1024, 6250] per core

Host merge (the all-gather + final top-k of the distributed ANN pattern):
  - per row, select every window whose max clears (128th-largest window max
    - MARGIN); gather those windows' 4 member columns as candidates
  - exact f64 re-rank of the candidates; final top-128 ordered like
    jax.lax.top_k (value desc, index asc)
  - exactness guard: MARGIN >= 2*EPS guarantees containment of the true
    top-128 given |device window max - exact window max| <= EPS; EPS is
    validated per-run on every selected window (device value vs exact f64
    value), and violating rows (expected none) are recomputed exactly.
"""

import numpy as np

B = 1024
D = 128
VOCAB = 200000
NCORES = 8
VSHARD = VOCAB // NCORES  # 25000
REGION = 2048  # 4 PSUM banks of f32
NREG = 12  # full regions per shard
TAIL = VSHARD - NREG * REGION  # 424
WINDOW = 4
NWIN = VSHARD // WINDOW  # 6250 window maxima per core per row
TOPK = 128

# Engine route per full region (tail is always 'A'):
#   A = DVE windowed reduce, B = Act copy + DVE folds, C = Pool pair + DVE fold
ROUTES = "BBABBBABBBAB"

# |device window max - exact window max| bound: bf16 matmul noise (~0.1) +
# bf16 output quantization (~0.2 at score ~45).  Validated at runtime.
EPS_BOUND = 0.5
MARGIN = 1.2  # >= 2*EPS_BOUND + slack

LAST_RESULTS = None  # BassKernelResults of the most recent run (for profiling)
_CACHED_NC = None


def build_kernel():
    import concourse.bass as bass  # noqa: F401
    import concourse.tile as tile
    from concourse import bacc, mybir

    F32 = mybir.dt.float32
    BF16 = mybir.dt.bfloat16
    AX = mybir.AxisListType.X
    MAX = mybir.AluOpType.max
    COPY = mybir.ActivationFunctionType.Copy

    nc = bacc.Bacc("TRN2", target_bir_lowering=False, debug=False)
    wt_d = nc.dram_tensor("wt", [D, VSHARD], BF16, kind="ExternalInput")
    xt_d = nc.dram_tensor("xt", [D, B], BF16, kind="ExternalInput")
    out_d = nc.dram_tensor("out_win", [B, NWIN], BF16, kind="ExternalOutput")

    with tile.TileContext(nc) as tc:
        with (
            tc.tile_pool(name="wt", bufs=1) as wt_pool,
            tc.tile_pool(name="xt", bufs=1) as xt_pool,
            tc.tile_pool(name="psum", bufs=2, space="PSUM") as psum_pool,
            tc.tile_pool(name="outw", bufs=2) as out_pool,
            tc.tile_pool(name="ac", bufs=2) as ac_pool,
            tc.tile_pool(name="fold", bufs=2) as fold_pool,
            tc.tile_pool(name="po", bufs=2) as po_pool,
        ):
            wt_sb = wt_pool.tile([D, VSHARD], BF16)
            xt_sb = xt_pool.tile([D, B], BF16)
            # xt first: the first matmul's stationary operand should not wait
            # behind the whole 6.4MB W load; 16 splits spread W across queues.
            nc.sync.dma_start(xt_sb[:], xt_d[:])
            nsplit = 16
            step = VSHARD // nsplit
            for s in range(nsplit):
                hi = VSHARD if s == nsplit - 1 else (s + 1) * step
                nc.sync.dma_start(wt_sb[:, s * step:hi], wt_d[:, s * step:hi])

            for g in range(B // 128):
                out_sb = out_pool.tile([128, NWIN], BF16, tag="outw")
                xg = xt_sb[:, g * 128:(g + 1) * 128]
                for r in range(NREG + 1):
                    base = r * REGION
                    w_cols = REGION if r < NREG else TAIL
                    route = ROUTES[r] if r < NREG else "A"
                    ps = psum_pool.tile([128, REGION], F32)
                    for k in range(0, w_cols, 512):
                        kw = min(512, w_cols - k)
                        nc.tensor.matmul(
                            ps[:, k:k + kw],
                            xg,
                            wt_sb[:, base + k:base + k + kw],
                            start=True, stop=True,
                        )
                    owin = out_sb[:, base // 4:base // 4 + w_cols // 4]
                    if route == "A":
                        nc.vector.tensor_reduce(
                            owin,
                            ps[:, :w_cols].rearrange("p (n w) -> p n w", w=4),
                            axis=AX, op=MAX,
                        )
                    elif route == "B":
                        ac = ac_pool.tile([128, REGION], BF16, tag="ac")
                        nc.scalar.activation(ac[:], ps[:], COPY)
                        fd = fold_pool.tile([128, REGION // 2], BF16, tag="fold")
                        nc.vector.tensor_max(fd[:], ac[:, :1024], ac[:, 1024:])
                        nc.vector.tensor_max(owin, fd[:, :512], fd[:, 512:])
                    else:  # "C"
                        ac = ac_pool.tile([128, REGION], BF16, tag="ac")
                        nc.scalar.activation(ac[:], ps[:], COPY)
                        po = po_pool.tile([128, REGION // 2], BF16, tag="po")
                        nc.gpsimd.tensor_max(po[:], ac[:, :1024], ac[:, 1024:])
                        nc.vector.tensor_max(owin, po[:, :512], po[:, 512:])
                nc.sync.dma_start(out_d[g * 128:(g + 1) * 128, :], out_sb[:])
    nc.compile()
    return nc


def _build_colmap() -> np.ndarray:
    """[NWIN, 4] int64: window id -> the 4 shard-local vocab columns it covers."""
    cm = np.empty((NWIN, 4), np.int64)
    wbase = 0
    for r in range(NREG + 1):
        base = r * REGION
        n = (REGION if r < NREG else TAIL) // 4
        route = ROUTES[r] if r < NREG else "A"
        j = np.arange(n)[:, None]
        if route == "A":
            cols = base + 4 * j + np.arange(4)[None, :]
        else:  # B and C share the fold mapping {j, j+512, j+1024, j+1536}
            cols = base + j + np.array([0, 512, 1024, 1536])[None, :]
        cm[wbase:wbase + n] = cols
        wbase += n
    assert wbase == NWIN
    return cm


_COLMAP = _build_colmap()


def _topk_rows(vals, gidx, k):
    """Per-row top-k ordered like jax.lax.top_k: value desc, index asc."""
    order = np.lexsort((gidx, -vals), axis=-1)[:, :k]
    return (
        np.take_along_axis(gidx, order, axis=1),
        np.take_along_axis(vals, order, axis=1),
    )


def kernel(x: np.ndarray, W: np.ndarray, topk) -> np.ndarray:
    global LAST_RESULTS, _CACHED_NC
    import os

    import ml_dtypes

    from concourse.bass_utils import run_bass_kernel_spmd

    assert x.shape == (B, D) and W.shape == (VOCAB, D)
    assert int(topk) == TOPK
    x = np.ascontiguousarray(np.asarray(x, dtype=np.float32))
    W = np.ascontiguousarray(np.asarray(W, dtype=np.float32))

    if _CACHED_NC is None:
        _CACHED_NC = build_kernel()
    nc = _CACHED_NC

    xt = np.ascontiguousarray(x.T).astype(ml_dtypes.bfloat16)
    in_maps = []
    for i in range(NCORES):
        wt_i = np.ascontiguousarray(
            W[i * VSHARD:(i + 1) * VSHARD].T
        ).astype(ml_dtypes.bfloat16)
        in_maps.append({"wt": wt_i, "xt": xt})

    LAST_RESULTS = run_bass_kernel_spmd(
        nc,
        in_maps,
        core_ids=list(range(NCORES)),
        trace=bool(int(os.environ.get("KERNEL_TRACE", "0"))),
    )
    results = LAST_RESULTS.results

    # [B, 8*NWIN] device window maxima, f32
    wm = np.concatenate(
        [np.asarray(results[i]["out_win"]).astype(np.float32)
         for i in range(NCORES)], axis=1,
    )
    nwin_all = NCORES * NWIN

    # Per-row window selection: everything >= (128th-largest window max - MARGIN)
    kth = np.partition(wm, nwin_all - TOPK, axis=1)[:, nwin_all - TOPK]
    tau = kth - MARGIN
    counts = (wm >= tau[:, None]).sum(axis=1)
    K = int(min(max(int(counts.max()), TOPK + 64), 4096))
    topw = np.argpartition(-wm, K - 1, axis=1)[:, :K]  # [B, K] window ids

    core_id = topw // NWIN
    wi = topw % NWIN
    cand = (_COLMAP[wi] + core_id[..., None] * VSHARD).reshape(B, K * 4)

    # Exact f64 re-rank of the candidate columns.
    x64 = x.astype(np.float64)
    W64 = W.astype(np.float64)
    exact = np.empty((B, K * 4), np.float64)
    STEP = 128
    for r0 in range(0, B, STEP):
        r1 = r0 + STEP
        gW = W64[cand[r0:r1]]  # [STEP, K*4, D]
        exact[r0:r1] = np.einsum("bjd,bd->bj", gW, x64[r0:r1])

    gidx_top, vals_top = _topk_rows(exact, cand, TOPK)

    # Exactness guards.
    t128 = vals_top[:, -1]
    dev_wmax = np.take_along_axis(wm, topw, axis=1)
    true_wmax = exact.reshape(B, K, 4).max(axis=2)
    err = np.abs(dev_wmax - true_wmax).max(axis=1)
    bad = (
        (err > EPS_BOUND)
        | (tau + EPS_BOUND > t128)
        | (counts > K)
    )
    if os.environ.get("KERNEL_DEBUG"):
        print(f"[kernel] K={K} counts max={counts.max()} "
              f"err max={err.max():.4f} bad rows={int(bad.sum())}")
    for r in np.flatnonzero(bad):
        s = x64[r] @ W64.T
        gidx_top[r] = np.lexsort((np.arange(VOCAB), -s))[:TOPK]

    return gidx_top.astype(np.int32)
